# revision 10
# baseline (speedup 1.0000x reference)
"""DeepSet segment-reduce kernel for 8 Trainium2 NeuronCores (Bass/Tile).

Math (reference):
    h  = relu(x1 @ W1 + b1) @ W2 + b2          # [E, 128]
    S  = segment_sum(h, seg)                   # [B, 128]
    mean = S / max(counts, 1)
    out  = mean @ W3 + b3

Because segment-sum is linear, only r = relu(x1 @ W1 + b1) needs per-edge
work:  segsum(h) = segsum(r) @ W2 + counts x b2.  So the kernel:

  host: repack edges so every segment starts on a 128-edge block boundary
        (pad columns use x_pad with W1.T x_pad + b1 < 0, so relu kills them
        exactly), shard segment runs across 8 cores balanced by block count.
  core: stream xT [128, E_cap] tiles; hT = W1.T @ xT (PE, W1 stationary);
        relu+bias on ACT (PSUM->SBUF); per-128-block sums on DVE (3-D
        tensor_reduce);  ST[f, b] = block_sumsT @ A  via PE transposes +
        matmuls where A [J, B] is the per-core one-hot block->segment map
        (an input tensor, so the instruction stream is identical on all
        cores -> one SPMD program);
        AllReduce partial ST over the 8 cores;
        sums_hT = W2.T @ S + b2 x counts (rank-1 via k=1 matmul);
        meanT   = sums_hT * inv_counts (broadcast tensor passed as input);
        out     = meanT.T @ W3 + 1 x b3 (rank-1), DMA to [B, 128] output.

Self-contained: no reads of /root/problem/*; shapes derived from inputs.
"""

import math

import numpy as np

N_CORES = 8
BLOCK = 128          # segment alignment quantum (edges per block)
DMA_TILE = 4096      # xT columns per DMA (2 MiB)
PSUM_TILE = 2048     # columns per PSUM tile / ACT op (4 banks)
PAD_MARGIN = 8.0


def _plan_shards(edge_slices, E, B):
    es = np.asarray(edge_slices, dtype=np.int64)
    counts = (es[1:] - es[:-1]).astype(np.int64)        # [B]
    seg_blocks = (counts + BLOCK - 1) // BLOCK          # [B]
    total_blocks = int(seg_blocks.sum())

    # contiguous runs of segments per core, balanced by block count
    cum = np.cumsum(seg_blocks)
    bounds = [0]
    for c in range(1, N_CORES):
        bounds.append(int(np.searchsorted(cum, c * total_blocks / N_CORES)))
    bounds.append(B)

    core_blocks = []
    for c in range(N_CORES):
        core_blocks.append(int(seg_blocks[bounds[c]:bounds[c + 1]].sum()))
    j_max = max(core_blocks)
    e_cap = int(math.ceil(j_max * BLOCK / DMA_TILE) * DMA_TILE)
    return es, counts, seg_blocks, bounds, e_cap


def _solve_xpad(W1, b1):
    # x_pad with W1.T x_pad + b1 = -PAD_MARGIN elementwise => relu output 0
    rhs = -(b1.astype(np.float64) + PAD_MARGIN)
    x_pad = np.linalg.solve(W1.astype(np.float64).T, rhs)
    chk = W1.astype(np.float64).T @ x_pad + b1.astype(np.float64)
    assert chk.max() < -1.0, "x_pad margin too small"
    return x_pad.astype(np.float32)


def _build_core_inputs(x1, es, counts, seg_blocks, bounds, e_cap, x_pad, B):
    J = e_cap // BLOCK
    xTs, As = [], []
    for c in range(N_CORES):
        xT = np.empty((128, e_cap), dtype=np.float32)
        A = np.zeros((J, B), dtype=np.float32)
        pos = 0
        for b in range(bounds[c], bounds[c + 1]):
            cnt = int(counts[b])
            if cnt == 0:
                continue
            xT[:, pos:pos + cnt] = x1[es[b]:es[b + 1], :].T
            nb = int(seg_blocks[b])
            pad = nb * BLOCK - cnt
            if pad:
                xT[:, pos + cnt:pos + nb * BLOCK] = x_pad[:, None]
            A[pos // BLOCK: pos // BLOCK + nb, b] = 1.0
            pos += nb * BLOCK
        if pos < e_cap:
            xT[:, pos:] = x_pad[:, None]
        xTs.append(xT)
        As.append(A)
    return xTs, As, J


def _build_bass(e_cap, J, B, collective=True):
    import concourse.bacc as bacc
    import concourse.mybir as mybir
    import concourse.tile as tile

    f32 = mybir.dt.float32
    Relu = mybir.ActivationFunctionType.Relu

    nc = bacc.Bacc(trn_type="TRN2", num_devices=N_CORES)

    xT_d = nc.dram_tensor("xT", [128, e_cap], f32, kind="ExternalInput")
    A_d = nc.dram_tensor("A", [J, B], f32, kind="ExternalInput")
    W1_d = nc.dram_tensor("W1", [128, 128], f32, kind="ExternalInput")
    b1_d = nc.dram_tensor("b1c", [128, 1], f32, kind="ExternalInput")
    W2_d = nc.dram_tensor("W2", [128, 128], f32, kind="ExternalInput")
    b2_d = nc.dram_tensor("b2r", [1, 128], f32, kind="ExternalInput")
    W3_d = nc.dram_tensor("W3", [128, 128], f32, kind="ExternalInput")
    b3_d = nc.dram_tensor("b3r", [1, 128], f32, kind="ExternalInput")
    cnt_d = nc.dram_tensor("counts_row", [1, B], f32, kind="ExternalInput")
    inv_d = nc.dram_tensor("inv_bcast", [128, B], f32, kind="ExternalInput")
    ones_d = nc.dram_tensor("ones_row", [1, 128], f32, kind="ExternalInput")
    ident_d = nc.dram_tensor("ident", [128, 128], f32, kind="ExternalInput")
    out_d = nc.dram_tensor("out", [B, 128], f32, kind="ExternalOutput")

    n_dma = e_cap // DMA_TILE
    n_half = DMA_TILE // PSUM_TILE
    blk_per_ps = PSUM_TILE // BLOCK
    n_chunks = (J + 127) // 128

    with tile.TileContext(nc) as tc, tc.tile_pool(name="persist", bufs=1) as pp:
        # persistent tiles (distinct tags -> own slot each)
        w1_sb = pp.tile([128, 128], f32, name="w1_sb")
        b1_sb = pp.tile([128, 1], f32, name="b1_sb")
        ident_sb = pp.tile([128, 128], f32, name="ident_sb")
        bs_sb = pp.tile([128, J], f32, name="bs_sb")
        bsT_sb = pp.tile([128, n_chunks * 128], f32, name="bsT_sb")
        nc.sync.dma_start(w1_sb[:], W1_d[:])
        nc.sync.dma_start(b1_sb[:], b1_d[:])
        nc.sync.dma_start(ident_sb[:], ident_d[:])

        # ---- main loop: hT = W1.T @ xT, relu+bias, per-block sums ----
        with (
            tc.tile_pool(name="xp", bufs=3) as xp,
            tc.tile_pool(name="rp", bufs=2) as rp,
            tc.tile_pool(name="hp", bufs=2, space="PSUM") as hp,
        ):
            for t in range(n_dma):
                xt = xp.tile([128, DMA_TILE], f32, name="xt")
                nc.sync.dma_start(xt[:], xT_d[:, t * DMA_TILE:(t + 1) * DMA_TILE])
                for h in range(n_half):
                    ps = hp.tile([128, PSUM_TILE], f32, name="ps")
                    for q in range(PSUM_TILE // 512):
                        c0 = h * PSUM_TILE + q * 512
                        # float32r: fp32 data in PE replay mode — 1 cyc/row
                        # at n=512 vs 4 cyc/row for plain fp32
                        nc.tensor.matmul(
                            ps[:, q * 512:(q + 1) * 512],
                            lhsT=w1_sb[:].bitcast(mybir.dt.float32r),
                            rhs=xt[:, c0:c0 + 512].bitcast(mybir.dt.float32r),
                            start=True, stop=True,
                        )
                    rt = rp.tile([128, PSUM_TILE], f32, name="rt")
                    nc.scalar.activation(rt[:], ps[:], Relu, bias=b1_sb[:, 0:1])
                    j0 = (t * DMA_TILE + h * PSUM_TILE) // BLOCK
                    nc.vector.tensor_reduce(
                        bs_sb[:, j0:j0 + blk_per_ps],
                        rt[:].rearrange("p (j e) -> p j e", e=BLOCK),
                        axis=mybir.AxisListType.X,
                        op=mybir.AluOpType.add,
                    )

        # ---- block sums -> per-core partial ST[f, b] ----
        with (
            tc.tile_pool(name="tp", bufs=2, space="PSUM") as tp,
            tc.tile_pool(name="ap", bufs=2) as ap_pool,
            tc.tile_pool(name="stp", bufs=1, space="PSUM") as stp,
        ):
            for k in range(n_chunks):
                jw = min(128, J - k * 128)
                tps = tp.tile([128, 128], f32, name="tps")
                nc.tensor.transpose(
                    tps[:jw, :], bs_sb[:, k * 128:k * 128 + jw], ident_sb[:]
                )
                nc.scalar.copy(bsT_sb[:jw, k * 128:(k + 1) * 128], tps[:jw, :])

            a_tiles = []
            for k in range(n_chunks):
                jw = min(128, J - k * 128)
                at = ap_pool.tile([128, B], f32, name="at", bufs=n_chunks)
                nc.sync.dma_start(at[:jw, :], A_d[k * 128:k * 128 + jw, :])
                a_tiles.append((at, jw))

            st_ps = stp.tile([128, B], f32, name="st_ps")
            for n0 in range(0, B, 512):
                nw = min(512, B - n0)
                for k in range(n_chunks):
                    at, jw = a_tiles[k]
                    nc.tensor.matmul(
                        st_ps[:, n0:n0 + nw],
                        lhsT=bsT_sb[:jw, k * 128:(k + 1) * 128],
                        rhs=at[:jw, n0:n0 + nw],
                        start=(k == 0), stop=(k == n_chunks - 1),
                    )
            st_sb = pp.tile([128, B], f32, name="st_sb")
            nc.scalar.copy(st_sb[:], st_ps[:])

        # ---- AllReduce partial ST across cores ----
        with tc.tile_pool(name="dram", bufs=1, space="DRAM") as dp:
            cc_in = dp.tile([128, B], f32, name="cc_in")
            cc_out = dp.tile([128, B], f32, name="cc_out", addr_space="Shared")
            nc.gpsimd.dma_start(cc_in[:], st_sb[:])
            if collective:
                nc.gpsimd.collective_compute(
                    "AllReduce",
                    mybir.AluOpType.add,
                    replica_groups=[list(range(N_CORES))],
                    ins=[cc_in.opt()],
                    outs=[cc_out.opt()],
                )
            else:  # timeline-sim variant: plain copy stands in for AllReduce
                nc.gpsimd.dma_start(cc_out[:], cc_in[:])
            sfull_sb = pp.tile([128, B], f32, name="sfull_sb")
            nc.gpsimd.dma_start(sfull_sb[:], cc_out[:])

        # ---- final: sums_hT = W2.T @ S + b2 x counts; meanT; out ----
        w2_sb = pp.tile([128, 128], f32, name="w2_sb")
        b2_sb = pp.tile([1, 128], f32, name="b2_sb")
        w3_sb = pp.tile([128, 128], f32, name="w3_sb")
        b3_sb = pp.tile([1, 128], f32, name="b3_sb")
        cnt_sb = pp.tile([1, B], f32, name="cnt_sb")
        inv_sb = pp.tile([128, B], f32, name="inv_sb")
        ones_sb = pp.tile([1, 128], f32, name="ones_sb")
        mean_sb = pp.tile([128, B], f32, name="mean_sb")
        nc.sync.dma_start(w2_sb[:], W2_d[:])
        nc.sync.dma_start(b2_sb[:], b2_d[:])
        nc.sync.dma_start(w3_sb[:], W3_d[:])
        nc.sync.dma_start(b3_sb[:], b3_d[:])
        nc.sync.dma_start(cnt_sb[:], cnt_d[:])
        nc.sync.dma_start(inv_sb[:], inv_d[:])
        nc.sync.dma_start(ones_sb[:], ones_d[:])

        with (
            tc.tile_pool(name="sp", bufs=1, space="PSUM") as sp,
            tc.tile_pool(name="op", bufs=2, space="PSUM") as op,
            tc.tile_pool(name="op_sb", bufs=8) as op_sb,
        ):
            sums_ps = sp.tile([128, B], f32, name="sums_ps")
            for n0 in range(0, B, 512):
                sl = slice(n0, min(n0 + 512, B))
                nc.tensor.matmul(sums_ps[:, sl], lhsT=w2_sb[:],
                                 rhs=sfull_sb[:, sl], start=True, stop=False)
                nc.tensor.matmul(sums_ps[:, sl], lhsT=b2_sb[0:1, :],
                                 rhs=cnt_sb[0:1, sl], start=False, stop=True)
            nc.vector.tensor_mul(mean_sb[:], sums_ps[:], inv_sb[:])

            for c0 in range(0, B, 128):
                cw = min(128, B - c0)
                ops = op.tile([128, 128], f32, name="ops")
                nc.tensor.matmul(ops[:cw, :], lhsT=mean_sb[:, c0:c0 + cw],
                                 rhs=w3_sb[:], start=True, stop=False)
                nc.tensor.matmul(ops[:cw, :], lhsT=ones_sb[0:1, :cw],
                                 rhs=b3_sb[0:1, :], start=False, stop=True)
                osb = op_sb.tile([128, 128], f32, name="osb")
                nc.scalar.copy(osb[:cw, :], ops[:cw, :])
                nc.sync.dma_start(out_d[c0:c0 + cw, :], osb[:cw, :])

    nc.compile()
    return nc


def _prepare(x1, edge_slices, W1, b1, W2, b2, W3, b3):
    """Host planning + per-core input construction + Bass program build."""
    x1 = np.ascontiguousarray(np.asarray(x1, dtype=np.float32))
    W1 = np.asarray(W1, dtype=np.float32)
    b1 = np.asarray(b1, dtype=np.float32)
    E = x1.shape[0]
    B = int(np.asarray(edge_slices).shape[0]) - 1

    es, counts, seg_blocks, bounds, e_cap = _plan_shards(edge_slices, E, B)
    x_pad = _solve_xpad(W1, b1)
    xTs, As, J = _build_core_inputs(x1, es, counts, seg_blocks, bounds,
                                    e_cap, x_pad, B)

    counts_f = counts.astype(np.float32)
    inv = (1.0 / np.maximum(counts_f, 1.0)).astype(np.float32)
    shared = {
        "W1": W1,
        "b1c": np.ascontiguousarray(b1.reshape(128, 1)),
        "W2": np.asarray(W2, dtype=np.float32),
        "b2r": np.ascontiguousarray(np.asarray(b2, np.float32).reshape(1, 128)),
        "W3": np.asarray(W3, dtype=np.float32),
        "b3r": np.ascontiguousarray(np.asarray(b3, np.float32).reshape(1, 128)),
        "counts_row": np.ascontiguousarray(counts_f.reshape(1, B)),
        "inv_bcast": np.ascontiguousarray(np.repeat(inv.reshape(1, B), 128, axis=0)),
        "ones_row": np.ones((1, 128), np.float32),
        "ident": np.eye(128, dtype=np.float32),
    }

    nc = _build_bass(e_cap, J, B)
    in_maps = [{"xT": xTs[c], "A": As[c], **shared} for c in range(N_CORES)]
    return nc, in_maps


def kernel(x1, edge_slices, W1, b1, W2, b2, W3, b3):
    from concourse import bass_utils

    nc, in_maps = _prepare(x1, edge_slices, W1, b1, W2, b2, W3, b3)
    br = bass_utils.run_bass_kernel_spmd(
        nc, in_maps, core_ids=list(range(N_CORES))
    )
    return br.results[0]["out"]
